# revision 15
# baseline (speedup 1.0000x reference)
"""Trainium2 Bass kernel for a 2-layer GCN with data-aware attention gate.

Math (per reference):
    src,dst = edges + self-loops; deg = bincount(dst); dinv = rsqrt(deg)
    norm = dinv[src]*dinv[dst]
    h1 = relu(segsum(norm * (x@W1)[src], dst) + b1)
    h2 = relu(segsum(norm * (h1@W2)[src], dst) + b2)
    out = h2 * sigmoid(h2@attn_w + attn_b)

Device strategy (8 NeuronCores, node/dst-sharded):
    norm factorizes: agg[d] = dinv[d] * sum_{e->d} (dinv[s] * T[s]).
    Per layer: each core computes T' = (dinv .* H) @ W for its node shard
    (x is shipped pre-transposed and pre-scaled, so phase 1 is a single
    matmul per 128-node window), AllGather of the bf16 T' table, then
    per-edge dma_gather of T' rows from HBM and PE one-hot
    selection-matrix matmuls accumulate 128-slot window segment sums in
    PSUM.  The one-hot S matrices are built in one batched DVE is_equal
    per window using stride-0 broadcast access patterns.
    Self-loop contributions never enter the edge stream: they are the
    core's own table rows, kept in SBUF and added during the flush
    (agg_full[d] = dinv[d]*(psum[d] + T'[d]) since T' rows carry dinv[s]).
    int16 gather indices only reach 32767, so 256B gather elements cover
    2 adjacent 128B rows (idx=s>>1) with a parity split choosing the
    rhs column offset inside the gathered element; both layers use
    64-column bf16 table rows (layer 2 zero-pads 32 cols) so they share
    one identical gather/index/one-hot plan.
"""

import sys

import numpy as np

_CONC = "/opt/trn_rl_repo"
if _CONC not in sys.path:
    sys.path.insert(0, _CONC)

# ---------------------------------------------------------------------------
# configuration
# ---------------------------------------------------------------------------


class Cfg:
    def __init__(self, N=50000, DIN=128, DH=64, DOUT=32, NC=8, WPC=49, WPG=7):
        self.N, self.DIN, self.DH, self.DOUT = N, DIN, DH, DOUT
        self.NC, self.WPC, self.WPG = NC, WPC, WPG
        assert WPC % WPG == 0
        self.G = WPC // WPG            # gather groups per core
        self.NPC = WPC * 128           # slots per core
        self.TOT = NC * self.NPC       # total slots
        assert self.TOT // 2 <= 32768  # pair indices fit int16
        assert self.N <= self.TOT - 2


FULL = Cfg()

# ---------------------------------------------------------------------------
# host-side graph prep
# ---------------------------------------------------------------------------


def _assign_slots(deg, cfg):
    """LPT-deal nodes into NC*WPC bins of <=128 slots, balancing edge load.
    Returns pos[node] -> global slot position."""
    import heapq

    nbins = cfg.NC * cfg.WPC
    cap = np.full(nbins, 128, np.int64)
    order = np.argsort(-deg, kind="stable")
    heap = [(0, b) for b in range(nbins)]
    heapq.heapify(heap)
    count = np.zeros(nbins, np.int64)
    pos = np.empty(cfg.N, np.int64)
    for n in order:
        load, b = heapq.heappop(heap)
        pos[n] = b * 128 + count[b]
        count[b] += 1
        if count[b] < cap[b]:
            heapq.heappush(heap, (load + int(deg[n]), b))
    return pos


def prep(x, edge_index, cfg):
    """Build per-core input arrays and the static (SPMD-uniform) chunk plan."""
    N, NC, WPC, WPG, G = cfg.N, cfg.NC, cfg.WPC, cfg.WPG, cfg.G
    NPC, DIN = cfg.NPC, cfg.DIN
    import ml_dtypes
    bf16 = ml_dtypes.bfloat16

    # degree/norm INCLUDE self-loops; the edge stream excludes them
    # (self-loop terms are added analytically in the flush).
    src = edge_index[0].astype(np.int64)
    dst = edge_index[1].astype(np.int64)
    deg = (np.bincount(dst, minlength=N) + 1).astype(np.float32)
    dinv = (1.0 / np.sqrt(np.maximum(deg, 1e-12))).astype(np.float32)

    pos = _assign_slots(deg, cfg)

    # per-core transposed & dinv-pre-scaled x shard + per-slot dinv
    x_t = np.zeros((NC, DIN, NPC), np.float32)
    dinv_slot = np.ones((NC, 128, WPC), np.float32)
    node_of = np.full(cfg.TOT, -1, np.int64)
    node_of[pos] = np.arange(N)
    xs = np.asarray(x, np.float32)
    for c in range(NC):
        seg = node_of[c * NPC:(c + 1) * NPC]
        m = seg >= 0
        xw = np.zeros((NPC, DIN), np.float32)
        xw[m] = xs[seg[m]] * dinv[seg[m], None]
        x_t[c] = xw.T
        dv = np.ones(NPC, np.float32)
        dv[m] = dinv[seg[m]]
        dinv_slot[c] = dv.reshape(WPC, 128).T

    # edge records (both layers share one plan: pair idx + parity split)
    s_pos = pos[src]
    d_pos = pos[dst]
    c_e = d_pos // NPC
    w_e = (d_pos % NPC) // 128          # window within core
    dval_e = (d_pos % 128 + 2).astype(np.float32)  # slot-in-window + 2
    h_e = (s_pos & 1).astype(np.int64)
    gidx_e = (s_pos >> 1).astype(np.int64)

    key_all = (c_e * WPC + w_e) * 2 + h_e
    order_e = np.argsort(key_all, kind="stable")
    ks = key_all[order_e]
    bounds = np.searchsorted(ks, np.arange(NC * WPC * 2 + 1))
    buckets = {}
    for key in range(NC * WPC * 2):
        lo, hi = bounds[key], bounds[key + 1]
        if hi > lo:
            buckets[key] = order_e[lo:hi]

    tgt = np.zeros((WPC, 2), np.int64)
    for w in range(WPC):
        for h in range(2):
            mx = max(len(buckets.get((c * WPC + w) * 2 + h, ()))
                     for c in range(NC))
            tgt[w, h] = int(np.ceil(max(mx, 1) / 128) * 128)

    seglen = np.zeros((G, 2), np.int64)
    for g in range(G):
        for h in range(2):
            seglen[g, h] = tgt[g * WPG:(g + 1) * WPG, h].sum()

    # idx layout: per-(group,split) gather segments (window-major runs).
    # dval layout: per-window slabs (split-major runs) so one batched DVE
    # is_equal builds the whole window's one-hot stack.
    icols = 0
    ioff, gcol = {}, np.zeros((WPC, 2), np.int64)
    for g in range(G):
        for h in range(2):
            ioff[(g, h)] = icols
            icols += int(seglen[g, h]) // 16
            c0 = 0
            for wl in range(WPG):
                w = g * WPG + wl
                gcol[w, h] = c0
                c0 += int(tgt[w, h]) // 128
    ccols = 0
    scol = np.zeros(WPC, np.int64)
    hoff = np.zeros((WPC, 2), np.int64)
    nchw = np.zeros(WPC, np.int64)
    for w in range(WPC):
        scol[w] = ccols
        o = 0
        for h in range(2):
            hoff[w, h] = o
            o += int(tgt[w, h]) // 128
        nchw[w] = o
        ccols += o

    idx_all = np.zeros((NC, 128, icols), np.int16)
    dval_all = np.full((NC, 128, ccols), -1.0, np.float32)
    for c in range(NC):
        for g in range(G):
            for h in range(2):
                n = int(seglen[g, h])
                gi = np.zeros(n, np.int64)
                p = 0
                for wl in range(WPG):
                    w = g * WPG + wl
                    es = buckets.get((c * WPC + w) * 2 + h, ())
                    gi[p:p + len(es)] = gidx_e[es]
                    p += int(tgt[w, h])
                io = ioff[(g, h)]
                wrapped = gi.reshape(n // 16, 16).T.astype(np.int16)
                idx_all[c, :, io:io + n // 16] = np.tile(wrapped, (8, 1))
        for w in range(WPC):
            for h in range(2):
                es = buckets.get((c * WPC + w) * 2 + h, ())
                nt = int(tgt[w, h])
                dv = np.full(nt, -1.0, np.float32)
                dv[:len(es)] = dval_e[es]
                co = int(scol[w] + hoff[w, h])
                dval_all[c, :, co:co + nt // 128] = (
                    dv.reshape(nt // 128, 128).T)

    plan = dict(tgt=tgt, seglen=seglen, ioff=ioff, gcol=gcol, scol=scol,
                hoff=hoff, nchw=nchw, icols=icols, ccols=ccols)
    host = dict(x_t=x_t.astype(bf16), dinv_slot=dinv_slot, idx_all=idx_all,
                dval_all=dval_all.astype(bf16), pos=pos)
    return plan, host


# ---------------------------------------------------------------------------
# device kernel
# ---------------------------------------------------------------------------


def build(cfg, plan):
    import concourse.bass as bass  # noqa: F401
    import concourse.mybir as mybir
    import concourse.tile as tile
    from concourse import bacc

    NC, WPC, WPG, G = cfg.NC, cfg.WPC, cfg.WPG, cfg.G
    NPC, TOT, DIN, DH, DOUT = cfg.NPC, cfg.TOT, cfg.DIN, cfg.DH, cfg.DOUT
    f32 = mybir.dt.float32
    bf16 = mybir.dt.bfloat16
    AF = mybir.ActivationFunctionType
    tgt, seglen = plan["tgt"], plan["seglen"]
    ioff, gcol = plan["ioff"], plan["gcol"]
    scol, hoff, nchw = plan["scol"], plan["hoff"], plan["nchw"]

    nc = bacc.Bacc(
        "TRN2", target_bir_lowering=False, debug=False,
        num_devices=NC, num_swdge_queues=4,
    )

    # I/O
    x_d = nc.dram_tensor("x_t", [DIN, NPC], bf16, kind="ExternalInput")
    w1_d = nc.dram_tensor("w1", [DIN, DH], bf16, kind="ExternalInput")
    w2_d = nc.dram_tensor("w2", [DH, DOUT], f32, kind="ExternalInput")
    b1_d = nc.dram_tensor("b1rep", [128, DH], f32, kind="ExternalInput")
    b2_d = nc.dram_tensor("b2rep", [128, DOUT], f32, kind="ExternalInput")
    aw_d = nc.dram_tensor("awrep", [128, DOUT], f32, kind="ExternalInput")
    ab_d = nc.dram_tensor("abcol", [128, 1], f32, kind="ExternalInput")
    dv_d = nc.dram_tensor("dinv_slot", [128, WPC], f32, kind="ExternalInput")
    id_d = nc.dram_tensor("ident", [128, 128], f32, kind="ExternalInput")
    gi_d = nc.dram_tensor("giota", [128, 128], bf16, kind="ExternalInput")
    ix_d = nc.dram_tensor("idx_all", [128, plan["icols"]], mybir.dt.int16,
                          kind="ExternalInput")
    dvl_d = nc.dram_tensor("dval_all", [128, plan["ccols"]], bf16,
                           kind="ExternalInput")
    out_d = nc.dram_tensor("out_sh", [NPC, DOUT], f32, kind="ExternalOutput")

    rg = [list(range(NC))]

    with tile.TileContext(nc) as tc:
        with tc.tile_pool(name="const", bufs=1) as cpool:
            def load(dram, shape, dt=f32, eng=None):
                t = cpool.tile(shape, dt, tag=dram.name, name=dram.name + "_s")
                (eng or nc.sync).dma_start(t[:], dram.ap())
                return t

            w1_s = load(w1_d, [DIN, DH], bf16)
            w2_s = load(w2_d, [DH, DOUT])
            b1_s = load(b1_d, [128, DH])
            b2_s = load(b2_d, [128, DOUT])
            aw_s = load(aw_d, [128, DOUT])
            ab_s = load(ab_d, [128, 1])
            dv_s = load(dv_d, [128, WPC])
            id_s = load(id_d, [128, 128])
            gi_s = load(gi_d, [128, 128], bf16)
            ix_s = load(ix_d, [128, plan["icols"]], mybir.dt.int16,
                        eng=nc.scalar)
            dvl_s = load(dvl_d, [128, plan["ccols"]], bf16, eng=nc.scalar)

            # fused self-loop + bias slabs: dinv*T' + b, read by the flushes
            t1sl = cpool.tile([128, WPC * DH], f32, tag="t1sl", name="t1sl")
            t2sl = cpool.tile([128, WPC * DOUT], f32, tag="t2sl", name="t2sl")

            with tc.tile_pool(name="dram", bufs=1, space="DRAM") as dpool:
                t1_shard = dpool.tile([NPC, DH], bf16, tag="t1s", name="t1s")
                t1_full = dpool.tile([TOT, DH], bf16, tag="t1f", name="t1f",
                                     addr_space="Shared")
                t2_shard = dpool.tile([NPC, DH], bf16, tag="t2s", name="t2s")
                t2_full = dpool.tile([TOT, DH], bf16, tag="t2f", name="t2f",
                                     addr_space="Shared")

                # ---- phase 1: T1' = (dinv .* x) @ W1 (x pre-scaled, pre-T)
                with (
                    tc.tile_pool(name="tf_in", bufs=2) as pin,
                    tc.tile_pool(name="tf_ps", bufs=2, space="PSUM") as pps,
                    tc.tile_pool(name="tf_sb", bufs=3) as psb,
                ):
                    for w in range(WPC):
                        xt = pin.tile([128, 128], bf16, tag="xt", name="xt")
                        nc.sync.dma_start(
                            xt[:], x_d.ap()[:, w * 128:(w + 1) * 128])
                        hp = pps.tile([128, DH], f32, tag="hp", name="hp")
                        nc.tensor.matmul(hp[:], lhsT=xt[:], rhs=w1_s[:],
                                         start=True, stop=True)
                        hs = psb.tile([128, DH], bf16, tag="hs", name="hs")
                        nc.vector.tensor_copy(hs[:], hp[:])
                        nc.vector.scalar_tensor_tensor(
                            out=t1sl[:, w * DH:(w + 1) * DH], in0=hp[:],
                            scalar=dv_s[:, w:w + 1], in1=b1_s[:],
                            op0=mybir.AluOpType.mult, op1=mybir.AluOpType.add)
                        nc.sync.dma_start(
                            t1_shard[w * 128:(w + 1) * 128, :], hs[:])

                # ---- AllGather layer-1 table
                nc.gpsimd.collective_compute(
                    "AllGather", mybir.AluOpType.bypass, replica_groups=rg,
                    ins=[t1_shard[:]], outs=[t1_full[:]],
                )

                # ---- aggregation: gather rows + one-hot matmul segment sums
                def aggregate(full, ncols, pass1_fn, pass2_fn, qctr=[0]):
                    fv = full.rearrange("(a b) d -> a (b d)", b=2)
                    WSPLIT = (WPG + 1) // 2  # window-aligned gather piece cut
                    with (
                        tc.tile_pool(name="gpool", bufs=4) as gp,
                        tc.tile_pool(name="spool", bufs=6) as sp,
                        tc.tile_pool(name="apsum", bufs=4, space="PSUM") as aps,
                    ):
                        for g in range(G):
                            # gathers: per (split, piece), window-aligned
                            gts = {}
                            for h in range(2):
                                io = ioff[(g, h)]
                                cmid = int(gcol[g * WPG + WSPLIT, h])
                                ctot = int(seglen[g, h]) // 128
                                for pi, (c0, c1) in enumerate(
                                        ((0, cmid), (cmid, ctot))):
                                    n = (c1 - c0) * 128
                                    gt = gp.tile([128, n], bf16,
                                                 tag=f"g{h}{pi}",
                                                 name=f"gt{h}{pi}")
                                    nc.gpsimd.dma_gather(
                                        out_ap=gt[:].rearrange(
                                            "p (c d) -> p c d", d=128),
                                        in_ap=fv,
                                        idxs_ap=ix_s[:, io + c0 * 8:
                                                     io + c1 * 8],
                                        num_idxs=n, num_idxs_reg=n,
                                        elem_size=128,
                                        queue_num=qctr[0] % 4,
                                        single_packet=False,
                                    )
                                    qctr[0] += 1
                                    gts[(h, pi)] = (gt, c0)
                            # one-hot builds for the whole group up front
                            svs = {}
                            for wl in range(WPG):
                                w = g * WPG + wl
                                nw = int(nchw[w])
                                sv = sp.tile([128, nw * 128], bf16,
                                             tag="S", name="S")
                                c0 = int(scol[w])
                                dv3 = (dvl_s[:, c0:c0 + nw]
                                       .unsqueeze(2)
                                       .broadcast_to((128, nw, 128)))
                                gi3 = (gi_s[:].unsqueeze(1)
                                       .broadcast_to((128, nw, 128)))
                                s3 = sv[:].rearrange("p (c d) -> p c d", d=128)
                                nc.vector.tensor_tensor(
                                    out=s3, in0=dv3, in1=gi3,
                                    op=mybir.AluOpType.is_equal)
                                svs[wl] = sv
                            # pass 1: PSUM accumulation + scale/relu
                            keep = {}
                            for wl in range(WPG):
                                w = g * WPG + wl
                                pi = 0 if wl < WSPLIT else 1
                                ps = aps.tile([128, ncols], f32, tag="agg",
                                              name="agg")
                                chunks = []
                                for h in range(2):
                                    chunks += [(h, k) for k in
                                               range(int(tgt[w, h]) // 128)]
                                for j, (h, k) in enumerate(chunks):
                                    sc = int(hoff[w, h]) + k
                                    gt, c0 = gts[(h, pi)]
                                    tcol = int(gcol[w, h]) - c0 + k
                                    base = tcol * 128 + h * DH
                                    nc.tensor.matmul(
                                        ps[:],
                                        lhsT=svs[wl][:, sc * 128:
                                                     (sc + 1) * 128],
                                        rhs=gt[:, base:base + ncols],
                                        start=(j == 0),
                                        stop=(j == len(chunks) - 1),
                                    )
                                keep[wl] = pass1_fn(w, ps)
                            # pass 2: table prep / output, off the PE stream
                            for wl in range(WPG):
                                pass2_fn(g * WPG + wl, keep[wl])

                # ---- layer-1 flush: h=relu(dinv*agg+b1); T2'=(dinv.*h)@W2
                with (
                    tc.tile_pool(name="fl_sb", bufs=8) as fsb,
                    tc.tile_pool(name="fl_ps", bufs=2, space="PSUM") as fps,
                ):
                    def flush1a(w, ps):
                        # v2 = dinv*(psum + T1') + b1, with dinv*T1'+b1
                        # prebuilt in t1sl (self-loop absorbed)
                        v2 = fsb.tile([128, DH], f32, tag="v2", name="v2")
                        nc.vector.scalar_tensor_tensor(
                            out=v2[:], in0=ps[:], scalar=dv_s[:, w:w + 1],
                            in1=t1sl[:, w * DH:(w + 1) * DH],
                            op0=mybir.AluOpType.mult, op1=mybir.AluOpType.add)
                        # dinv*relu(x) == relu(dinv*x) since dinv>0
                        h2 = fsb.tile([128, DH], f32, tag="h2", name="h2")
                        nc.scalar.activation(h2[:], v2[:], func=AF.Relu,
                                             scale=dv_s[:, w:w + 1])
                        return h2

                    def flush1b(w, h2):
                        htp = fps.tile([DH, 128], f32, tag="htp", name="htp")
                        nc.tensor.transpose(htp[:], h2[:], id_s[:])
                        hts = fsb.tile([DH, 128], f32, tag="hts", name="hts")
                        nc.vector.tensor_copy(hts[:], htp[:])
                        t2p = fps.tile([128, DOUT], f32, tag="t2p", name="t2p")
                        nc.tensor.matmul(t2p[:], lhsT=hts[:], rhs=w2_s[:],
                                         start=True, stop=True)
                        t2b = fsb.tile([128, DH], bf16, tag="t2b",
                                       name="t2b")
                        nc.vector.memset(t2b[:, DOUT:], 0.0)
                        nc.vector.tensor_copy(t2b[:, :DOUT], t2p[:])
                        nc.vector.scalar_tensor_tensor(
                            out=t2sl[:, w * DOUT:(w + 1) * DOUT], in0=t2p[:],
                            scalar=dv_s[:, w:w + 1], in1=b2_s[:],
                            op0=mybir.AluOpType.mult, op1=mybir.AluOpType.add)
                        nc.sync.dma_start(
                            t2_shard[w * 128:(w + 1) * 128, :], t2b[:])

                    aggregate(t1_full[:], DH, flush1a, flush1b)

                    # ---- AllGather layer-2 table
                    nc.gpsimd.collective_compute(
                        "AllGather", mybir.AluOpType.bypass, replica_groups=rg,
                        ins=[t2_shard[:]], outs=[t2_full[:]],
                    )

                    # ---- layer-2 flush: h2 + attention gate -> out
                    def flush2a(w, ps):
                        v2 = fsb.tile([128, DOUT], f32, tag="f2v2",
                                      name="f2v2")
                        nc.vector.scalar_tensor_tensor(
                            out=v2[:], in0=ps[:], scalar=dv_s[:, w:w + 1],
                            in1=t2sl[:, w * DOUT:(w + 1) * DOUT],
                            op0=mybir.AluOpType.mult, op1=mybir.AluOpType.add)
                        hh = fsb.tile([128, DOUT], f32, tag="f2h", name="f2h")
                        nc.scalar.activation(hh[:], v2[:], func=AF.Relu)
                        return hh

                    def flush2b(w, hh):
                        a = fsb.tile([128, DOUT], f32, tag="f2a", name="f2a")
                        nc.vector.tensor_mul(out=a[:], in0=hh[:], in1=aw_s[:])
                        ar = fsb.tile([128, 1], f32, tag="f2ar", name="f2ar")
                        nc.vector.tensor_reduce(
                            ar[:], a[:], axis=mybir.AxisListType.X,
                            op=mybir.AluOpType.add)
                        at = fsb.tile([128, 1], f32, tag="f2at", name="f2at")
                        nc.scalar.activation(at[:], ar[:], func=AF.Sigmoid,
                                             bias=ab_s[:, :1])
                        o = fsb.tile([128, DOUT], f32, tag="f2o", name="f2o")
                        nc.scalar.activation(o[:], hh[:], func=AF.Copy,
                                             scale=at[:])
                        nc.sync.dma_start(
                            out_d.ap()[w * 128:(w + 1) * 128, :], o[:])

                    aggregate(t2_full[:], DOUT, flush2a, flush2b)

    nc.compile()
    return nc


# ---------------------------------------------------------------------------
# entry point
# ---------------------------------------------------------------------------


def _make_in_maps(cfg, host, W1, b1, W2, b2, attn_w, attn_b):
    import ml_dtypes
    NC = cfg.NC
    bf16 = ml_dtypes.bfloat16
    ident = np.eye(128, dtype=np.float32)
    giota = np.tile(np.arange(2, 130, dtype=np.float32), (128, 1)).astype(bf16)
    in_maps = []
    for c in range(NC):
        in_maps.append({
            "x_t": host["x_t"][c],
            "w1": np.asarray(W1, np.float32).astype(bf16),
            "w2": np.asarray(W2, np.float32),
            "b1rep": np.tile(np.asarray(b1, np.float32), (128, 1)),
            "b2rep": np.tile(np.asarray(b2, np.float32), (128, 1)),
            "awrep": np.tile(np.asarray(attn_w, np.float32).reshape(1, -1),
                             (128, 1)),
            "abcol": np.full((128, 1),
                             np.asarray(attn_b, np.float32).reshape(-1)[0],
                             np.float32),
            "dinv_slot": host["dinv_slot"][c],
            "ident": ident,
            "giota": giota,
            "idx_all": host["idx_all"][c],
            "dval_all": host["dval_all"][c],
        })
    return in_maps


def run(x, edge_index, W1, b1, W2, b2, attn_w, attn_b, cfg=None,
        backend="hw", trace=False):
    cfg = cfg or FULL
    plan, host = prep(x, edge_index, cfg)
    nc = build(cfg, plan)
    in_maps = _make_in_maps(cfg, host, W1, b1, W2, b2, attn_w, attn_b)

    if backend == "sim":
        from concourse.bass_interp import MultiCoreSim
        sim = MultiCoreSim(nc, num_cores=cfg.NC, trace=False)
        for c, core in enumerate(sim.cores.values()):
            for name, arr in in_maps[c].items():
                core.tensor(name)[:] = arr
        sim.simulate()
        outs = [core.tensor("out_sh").copy() for core in sim.cores.values()]
        exec_ns = None
    else:
        from concourse import bass_utils
        from concourse.bass_interp import get_hw_module
        old = nc.m
        nc.m = get_hw_module(nc.m)
        try:
            res = bass_utils.run_bass_kernel_spmd(
                nc, in_maps, core_ids=list(range(cfg.NC)), trace=trace)
        finally:
            nc.m = old
        outs = [res.results[c]["out_sh"] for c in range(cfg.NC)]
        exec_ns = res.exec_time_ns

    full = np.concatenate(outs, axis=0)  # [TOT, DOUT] in slot order
    out = full[host["pos"]]              # unpermute -> [N, DOUT]
    return np.ascontiguousarray(out), exec_ns


def kernel(x, edge_index, W1, b1, W2, b2, attn_w, attn_b):
    out, _ = run(x, edge_index, W1, b1, W2, b2, attn_w, attn_b,
                 cfg=FULL, backend="hw", trace=False)
    return out


# revision 16
# speedup vs baseline: 1.0414x; 1.0414x over previous
"""Trainium2 Bass kernel for a 2-layer GCN with data-aware attention gate.

Math (per reference):
    src,dst = edges + self-loops; deg = bincount(dst); dinv = rsqrt(deg)
    norm = dinv[src]*dinv[dst]
    h1 = relu(segsum(norm * (x@W1)[src], dst) + b1)
    h2 = relu(segsum(norm * (h1@W2)[src], dst) + b2)
    out = h2 * sigmoid(h2@attn_w + attn_b)

Device strategy (8 NeuronCores, node/dst-sharded):
    norm factorizes: agg[d] = dinv[d] * sum_{e->d} (dinv[s] * T[s]).
    Per layer: each core computes T' = (dinv .* H) @ W for its node shard
    (x is shipped pre-transposed and pre-scaled, so phase 1 is a single
    matmul per 128-node window), AllGather of the bf16 T' table, then
    per-edge dma_gather of T' rows from HBM and PE one-hot
    selection-matrix matmuls accumulate 128-slot window segment sums in
    PSUM.  The one-hot S matrices are built in one batched DVE is_equal
    per window using stride-0 broadcast access patterns.
    Self-loop contributions never enter the edge stream: they are the
    core's own table rows, kept in SBUF and added during the flush
    (agg_full[d] = dinv[d]*(psum[d] + T'[d]) since T' rows carry dinv[s]).
    int16 gather indices only reach 32767, so 256B gather elements cover
    2 adjacent 128B rows (idx=s>>1) with a parity split choosing the
    rhs column offset inside the gathered element; both layers use
    64-column bf16 table rows (layer 2 zero-pads 32 cols) so they share
    one identical gather/index/one-hot plan.
"""

import sys

import numpy as np

_CONC = "/opt/trn_rl_repo"
if _CONC not in sys.path:
    sys.path.insert(0, _CONC)

# ---------------------------------------------------------------------------
# configuration
# ---------------------------------------------------------------------------


class Cfg:
    def __init__(self, N=50000, DIN=128, DH=64, DOUT=32, NC=8, WPC=49, WPG=7):
        self.N, self.DIN, self.DH, self.DOUT = N, DIN, DH, DOUT
        self.NC, self.WPC, self.WPG = NC, WPC, WPG
        assert WPC % WPG == 0
        self.G = WPC // WPG            # gather groups per core
        self.NPC = WPC * 128           # slots per core
        self.TOT = NC * self.NPC       # total slots
        assert self.TOT // 2 <= 32768  # pair indices fit int16
        assert self.N <= self.TOT - 2


FULL = Cfg()

# ---------------------------------------------------------------------------
# host-side graph prep
# ---------------------------------------------------------------------------


def _assign_slots(deg, cfg):
    """LPT-deal nodes into NC*WPC bins of <=128 slots, balancing edge load.
    Returns pos[node] -> global slot position."""
    import heapq

    nbins = cfg.NC * cfg.WPC
    cap = np.full(nbins, 128, np.int64)
    order = np.argsort(-deg, kind="stable")
    heap = [(0, b) for b in range(nbins)]
    heapq.heapify(heap)
    count = np.zeros(nbins, np.int64)
    pos = np.empty(cfg.N, np.int64)
    for n in order:
        load, b = heapq.heappop(heap)
        pos[n] = b * 128 + count[b]
        count[b] += 1
        if count[b] < cap[b]:
            heapq.heappush(heap, (load + int(deg[n]), b))
    return pos


def prep(x, edge_index, cfg):
    """Build per-core input arrays and the static (SPMD-uniform) chunk plan."""
    N, NC, WPC, WPG, G = cfg.N, cfg.NC, cfg.WPC, cfg.WPG, cfg.G
    NPC, DIN = cfg.NPC, cfg.DIN
    import ml_dtypes
    bf16 = ml_dtypes.bfloat16

    # degree/norm INCLUDE self-loops; the edge stream excludes them
    # (self-loop terms are added analytically in the flush).
    src = edge_index[0].astype(np.int64)
    dst = edge_index[1].astype(np.int64)
    deg = (np.bincount(dst, minlength=N) + 1).astype(np.float32)
    dinv = (1.0 / np.sqrt(np.maximum(deg, 1e-12))).astype(np.float32)

    pos = _assign_slots(deg, cfg)

    # per-core transposed & dinv-pre-scaled x shard + per-slot dinv
    x_t = np.zeros((NC, DIN, NPC), np.float32)
    dinv_slot = np.ones((NC, 128, WPC), np.float32)
    node_of = np.full(cfg.TOT, -1, np.int64)
    node_of[pos] = np.arange(N)
    xs = np.asarray(x, np.float32)
    for c in range(NC):
        seg = node_of[c * NPC:(c + 1) * NPC]
        m = seg >= 0
        xw = np.zeros((NPC, DIN), np.float32)
        xw[m] = xs[seg[m]] * dinv[seg[m], None]
        x_t[c] = xw.T
        dv = np.ones(NPC, np.float32)
        dv[m] = dinv[seg[m]]
        dinv_slot[c] = dv.reshape(WPC, 128).T

    # edge records (both layers share one plan: pair idx + parity split)
    s_pos = pos[src]
    d_pos = pos[dst]
    c_e = d_pos // NPC
    w_e = (d_pos % NPC) // 128          # window within core
    dval_e = (d_pos % 128 + 2).astype(np.float32)  # slot-in-window + 2
    h_e = (s_pos & 1).astype(np.int64)
    gidx_e = (s_pos >> 1).astype(np.int64)

    key_all = (c_e * WPC + w_e) * 2 + h_e
    order_e = np.argsort(key_all, kind="stable")
    ks = key_all[order_e]
    bounds = np.searchsorted(ks, np.arange(NC * WPC * 2 + 1))
    buckets = {}
    for key in range(NC * WPC * 2):
        lo, hi = bounds[key], bounds[key + 1]
        if hi > lo:
            buckets[key] = order_e[lo:hi]

    tgt = np.zeros((WPC, 2), np.int64)
    for w in range(WPC):
        for h in range(2):
            mx = max(len(buckets.get((c * WPC + w) * 2 + h, ()))
                     for c in range(NC))
            tgt[w, h] = int(np.ceil(max(mx, 1) / 128) * 128)

    seglen = np.zeros((G, 2), np.int64)
    for g in range(G):
        for h in range(2):
            seglen[g, h] = tgt[g * WPG:(g + 1) * WPG, h].sum()

    # idx layout: per-(group,split) gather segments (window-major runs).
    # dval layout: per-window slabs (split-major runs) so one batched DVE
    # is_equal builds the whole window's one-hot stack.
    icols = 0
    ioff, gcol = {}, np.zeros((WPC, 2), np.int64)
    for g in range(G):
        for h in range(2):
            ioff[(g, h)] = icols
            icols += int(seglen[g, h]) // 16
            c0 = 0
            for wl in range(WPG):
                w = g * WPG + wl
                gcol[w, h] = c0
                c0 += int(tgt[w, h]) // 128
    ccols = 0
    scol = np.zeros(WPC, np.int64)
    hoff = np.zeros((WPC, 2), np.int64)
    nchw = np.zeros(WPC, np.int64)
    for w in range(WPC):
        scol[w] = ccols
        o = 0
        for h in range(2):
            hoff[w, h] = o
            o += int(tgt[w, h]) // 128
        nchw[w] = o
        ccols += o

    idx_all = np.zeros((NC, 128, icols), np.int16)
    dval_all = np.full((NC, 128, ccols), -1.0, np.float32)
    for c in range(NC):
        for g in range(G):
            for h in range(2):
                n = int(seglen[g, h])
                gi = np.zeros(n, np.int64)
                p = 0
                for wl in range(WPG):
                    w = g * WPG + wl
                    es = buckets.get((c * WPC + w) * 2 + h, ())
                    gi[p:p + len(es)] = gidx_e[es]
                    p += int(tgt[w, h])
                io = ioff[(g, h)]
                wrapped = gi.reshape(n // 16, 16).T.astype(np.int16)
                idx_all[c, :, io:io + n // 16] = np.tile(wrapped, (8, 1))
        for w in range(WPC):
            for h in range(2):
                es = buckets.get((c * WPC + w) * 2 + h, ())
                nt = int(tgt[w, h])
                dv = np.full(nt, -1.0, np.float32)
                dv[:len(es)] = dval_e[es]
                co = int(scol[w] + hoff[w, h])
                dval_all[c, :, co:co + nt // 128] = (
                    dv.reshape(nt // 128, 128).T)

    plan = dict(tgt=tgt, seglen=seglen, ioff=ioff, gcol=gcol, scol=scol,
                hoff=hoff, nchw=nchw, icols=icols, ccols=ccols)
    host = dict(x_t=x_t.astype(bf16), dinv_slot=dinv_slot, idx_all=idx_all,
                dval_all=dval_all.astype(bf16), pos=pos)
    return plan, host


# ---------------------------------------------------------------------------
# device kernel
# ---------------------------------------------------------------------------


def build(cfg, plan):
    import concourse.bass as bass  # noqa: F401
    import concourse.mybir as mybir
    import concourse.tile as tile
    from concourse import bacc

    NC, WPC, WPG, G = cfg.NC, cfg.WPC, cfg.WPG, cfg.G
    NPC, TOT, DIN, DH, DOUT = cfg.NPC, cfg.TOT, cfg.DIN, cfg.DH, cfg.DOUT
    f32 = mybir.dt.float32
    bf16 = mybir.dt.bfloat16
    AF = mybir.ActivationFunctionType
    tgt, seglen = plan["tgt"], plan["seglen"]
    ioff, gcol = plan["ioff"], plan["gcol"]
    scol, hoff, nchw = plan["scol"], plan["hoff"], plan["nchw"]

    nc = bacc.Bacc(
        "TRN2", target_bir_lowering=False, debug=False,
        num_devices=NC, num_swdge_queues=4,
    )

    # I/O
    x_d = nc.dram_tensor("x_t", [DIN, NPC], bf16, kind="ExternalInput")
    w1_d = nc.dram_tensor("w1", [DIN, DH], bf16, kind="ExternalInput")
    w2_d = nc.dram_tensor("w2", [DH, DOUT], f32, kind="ExternalInput")
    b1_d = nc.dram_tensor("b1rep", [128, DH], f32, kind="ExternalInput")
    b2_d = nc.dram_tensor("b2rep", [128, DOUT], f32, kind="ExternalInput")
    aw_d = nc.dram_tensor("awrep", [128, DOUT], f32, kind="ExternalInput")
    ab_d = nc.dram_tensor("abcol", [128, 1], f32, kind="ExternalInput")
    dv_d = nc.dram_tensor("dinv_slot", [128, WPC], f32, kind="ExternalInput")
    id_d = nc.dram_tensor("ident", [128, 128], f32, kind="ExternalInput")
    gi_d = nc.dram_tensor("giota", [128, 128], bf16, kind="ExternalInput")
    ix_d = nc.dram_tensor("idx_all", [128, plan["icols"]], mybir.dt.int16,
                          kind="ExternalInput")
    dvl_d = nc.dram_tensor("dval_all", [128, plan["ccols"]], bf16,
                           kind="ExternalInput")
    out_d = nc.dram_tensor("out_sh", [NPC, DOUT], f32, kind="ExternalOutput")

    rg = [list(range(NC))]

    with tile.TileContext(nc) as tc:
        with tc.tile_pool(name="const", bufs=1) as cpool:
            def load(dram, shape, dt=f32, eng=None):
                t = cpool.tile(shape, dt, tag=dram.name, name=dram.name + "_s")
                (eng or nc.sync).dma_start(t[:], dram.ap())
                return t

            w1_s = load(w1_d, [DIN, DH], bf16)
            w2_s = load(w2_d, [DH, DOUT])
            b1_s = load(b1_d, [128, DH])
            b2_s = load(b2_d, [128, DOUT])
            aw_s = load(aw_d, [128, DOUT])
            ab_s = load(ab_d, [128, 1])
            dv_s = load(dv_d, [128, WPC])
            id_s = load(id_d, [128, 128])
            gi_s = load(gi_d, [128, 128], bf16)
            ix_s = load(ix_d, [128, plan["icols"]], mybir.dt.int16,
                        eng=nc.scalar)
            dvl_s = load(dvl_d, [128, plan["ccols"]], bf16, eng=nc.scalar)

            # fused self-loop + bias slabs: dinv*T' + b, read by the flushes
            t1sl = cpool.tile([128, WPC * DH], f32, tag="t1sl", name="t1sl")
            t2sl = cpool.tile([128, WPC * DOUT], f32, tag="t2sl", name="t2sl")

            with tc.tile_pool(name="dram", bufs=1, space="DRAM") as dpool:
                t1_shard = dpool.tile([NPC, DH], bf16, tag="t1s", name="t1s")
                t1_full = dpool.tile([TOT, DH], bf16, tag="t1f", name="t1f",
                                     addr_space="Shared")
                t2_shard = dpool.tile([NPC, DH], bf16, tag="t2s", name="t2s")
                t2_full = dpool.tile([TOT, DH], bf16, tag="t2f", name="t2f",
                                     addr_space="Shared")

                # ---- phase 1: T1' = (dinv .* x) @ W1 (x pre-scaled, pre-T)
                with (
                    tc.tile_pool(name="tf_in", bufs=2) as pin,
                    tc.tile_pool(name="tf_ps", bufs=2, space="PSUM") as pps,
                    tc.tile_pool(name="tf_sb", bufs=3) as psb,
                ):
                    for w in range(WPC):
                        xt = pin.tile([128, 128], bf16, tag="xt", name="xt")
                        nc.sync.dma_start(
                            xt[:], x_d.ap()[:, w * 128:(w + 1) * 128])
                        hp = pps.tile([128, DH], f32, tag="hp", name="hp")
                        nc.tensor.matmul(hp[:], lhsT=xt[:], rhs=w1_s[:],
                                         start=True, stop=True)
                        hs = psb.tile([128, DH], bf16, tag="hs", name="hs")
                        nc.vector.tensor_copy(hs[:], hp[:])
                        nc.vector.scalar_tensor_tensor(
                            out=t1sl[:, w * DH:(w + 1) * DH], in0=hp[:],
                            scalar=dv_s[:, w:w + 1], in1=b1_s[:],
                            op0=mybir.AluOpType.mult, op1=mybir.AluOpType.add)
                        nc.sync.dma_start(
                            t1_shard[w * 128:(w + 1) * 128, :], hs[:])

                # ---- AllGather layer-1 table
                nc.gpsimd.collective_compute(
                    "AllGather", mybir.AluOpType.bypass, replica_groups=rg,
                    ins=[t1_shard[:]], outs=[t1_full[:]],
                )

                # ---- aggregation: gather rows + one-hot matmul segment sums
                def aggregate(full, ncols, pass1_fn, pass2_fn, qctr=[0]):
                    fv = full.rearrange("(a b) d -> a (b d)", b=2)
                    # window-aligned gather pieces (~2 windows each)
                    PCUTS = list(range(0, WPG, 2)) + [WPG]
                    with (
                        tc.tile_pool(name="gpool", bufs=4) as gp,
                        tc.tile_pool(name="spool", bufs=6) as sp,
                        tc.tile_pool(name="apsum", bufs=4, space="PSUM") as aps,
                    ):
                        for g in range(G):
                            # gathers: per (split, piece), window-aligned
                            gts = {}
                            for h in range(2):
                                io = ioff[(g, h)]
                                ctot = int(seglen[g, h]) // 128
                                cb = [int(gcol[g * WPG + wc, h])
                                      if wc < WPG else ctot for wc in PCUTS]
                                for pi, (c0, c1) in enumerate(
                                        zip(cb[:-1], cb[1:])):
                                    n = (c1 - c0) * 128
                                    gt = gp.tile([128, n], bf16,
                                                 tag=f"g{h}{pi}",
                                                 name=f"gt{h}{pi}")
                                    nc.gpsimd.dma_gather(
                                        out_ap=gt[:].rearrange(
                                            "p (c d) -> p c d", d=128),
                                        in_ap=fv,
                                        idxs_ap=ix_s[:, io + c0 * 8:
                                                     io + c1 * 8],
                                        num_idxs=n, num_idxs_reg=n,
                                        elem_size=128,
                                        queue_num=qctr[0] % 4,
                                        single_packet=False,
                                    )
                                    qctr[0] += 1
                                    gts[(h, pi)] = (gt, c0)
                            # one-hot builds for the whole group up front
                            svs = {}
                            for wl in range(WPG):
                                w = g * WPG + wl
                                nw = int(nchw[w])
                                sv = sp.tile([128, nw * 128], bf16,
                                             tag="S", name="S")
                                c0 = int(scol[w])
                                dv3 = (dvl_s[:, c0:c0 + nw]
                                       .unsqueeze(2)
                                       .broadcast_to((128, nw, 128)))
                                gi3 = (gi_s[:].unsqueeze(1)
                                       .broadcast_to((128, nw, 128)))
                                s3 = sv[:].rearrange("p (c d) -> p c d", d=128)
                                nc.vector.tensor_tensor(
                                    out=s3, in0=dv3, in1=gi3,
                                    op=mybir.AluOpType.is_equal)
                                svs[wl] = sv
                            # pass 1: PSUM accumulation + scale/relu
                            keep = {}
                            for wl in range(WPG):
                                w = g * WPG + wl
                                pi = wl // 2
                                ps = aps.tile([128, ncols], f32, tag="agg",
                                              name="agg")
                                chunks = []
                                for h in range(2):
                                    chunks += [(h, k) for k in
                                               range(int(tgt[w, h]) // 128)]
                                for j, (h, k) in enumerate(chunks):
                                    sc = int(hoff[w, h]) + k
                                    gt, c0 = gts[(h, pi)]
                                    tcol = int(gcol[w, h]) - c0 + k
                                    base = tcol * 128 + h * DH
                                    nc.tensor.matmul(
                                        ps[:],
                                        lhsT=svs[wl][:, sc * 128:
                                                     (sc + 1) * 128],
                                        rhs=gt[:, base:base + ncols],
                                        start=(j == 0),
                                        stop=(j == len(chunks) - 1),
                                    )
                                keep[wl] = pass1_fn(w, ps)
                            # pass 2: table prep / output, off the PE stream
                            for wl in range(WPG):
                                pass2_fn(g * WPG + wl, keep[wl])

                # ---- layer-1 flush: h=relu(dinv*agg+b1); T2'=(dinv.*h)@W2
                with (
                    tc.tile_pool(name="fl_sb", bufs=8) as fsb,
                    tc.tile_pool(name="fl_ps", bufs=2, space="PSUM") as fps,
                ):
                    def flush1a(w, ps):
                        # v2 = dinv*(psum + T1') + b1, with dinv*T1'+b1
                        # prebuilt in t1sl (self-loop absorbed)
                        v2 = fsb.tile([128, DH], f32, tag="v2", name="v2")
                        nc.vector.scalar_tensor_tensor(
                            out=v2[:], in0=ps[:], scalar=dv_s[:, w:w + 1],
                            in1=t1sl[:, w * DH:(w + 1) * DH],
                            op0=mybir.AluOpType.mult, op1=mybir.AluOpType.add)
                        # dinv*relu(x) == relu(dinv*x) since dinv>0
                        h2 = fsb.tile([128, DH], f32, tag="h2", name="h2")
                        nc.scalar.activation(h2[:], v2[:], func=AF.Relu,
                                             scale=dv_s[:, w:w + 1])
                        return h2

                    def flush1b(w, h2):
                        htp = fps.tile([DH, 128], f32, tag="htp", name="htp")
                        nc.tensor.transpose(htp[:], h2[:], id_s[:])
                        hts = fsb.tile([DH, 128], f32, tag="hts", name="hts")
                        nc.vector.tensor_copy(hts[:], htp[:])
                        t2p = fps.tile([128, DOUT], f32, tag="t2p", name="t2p")
                        nc.tensor.matmul(t2p[:], lhsT=hts[:], rhs=w2_s[:],
                                         start=True, stop=True)
                        t2b = fsb.tile([128, DH], bf16, tag="t2b",
                                       name="t2b")
                        nc.vector.memset(t2b[:, DOUT:], 0.0)
                        nc.vector.tensor_copy(t2b[:, :DOUT], t2p[:])
                        nc.vector.scalar_tensor_tensor(
                            out=t2sl[:, w * DOUT:(w + 1) * DOUT], in0=t2p[:],
                            scalar=dv_s[:, w:w + 1], in1=b2_s[:],
                            op0=mybir.AluOpType.mult, op1=mybir.AluOpType.add)
                        nc.sync.dma_start(
                            t2_shard[w * 128:(w + 1) * 128, :], t2b[:])

                    aggregate(t1_full[:], DH, flush1a, flush1b)

                    # ---- AllGather layer-2 table
                    nc.gpsimd.collective_compute(
                        "AllGather", mybir.AluOpType.bypass, replica_groups=rg,
                        ins=[t2_shard[:]], outs=[t2_full[:]],
                    )

                    # ---- layer-2 flush: h2 + attention gate -> out
                    def flush2a(w, ps):
                        v2 = fsb.tile([128, DOUT], f32, tag="f2v2",
                                      name="f2v2")
                        nc.vector.scalar_tensor_tensor(
                            out=v2[:], in0=ps[:], scalar=dv_s[:, w:w + 1],
                            in1=t2sl[:, w * DOUT:(w + 1) * DOUT],
                            op0=mybir.AluOpType.mult, op1=mybir.AluOpType.add)
                        hh = fsb.tile([128, DOUT], f32, tag="f2h", name="f2h")
                        nc.scalar.activation(hh[:], v2[:], func=AF.Relu)
                        return hh

                    def flush2b(w, hh):
                        a = fsb.tile([128, DOUT], f32, tag="f2a", name="f2a")
                        nc.vector.tensor_mul(out=a[:], in0=hh[:], in1=aw_s[:])
                        ar = fsb.tile([128, 1], f32, tag="f2ar", name="f2ar")
                        nc.vector.tensor_reduce(
                            ar[:], a[:], axis=mybir.AxisListType.X,
                            op=mybir.AluOpType.add)
                        at = fsb.tile([128, 1], f32, tag="f2at", name="f2at")
                        nc.scalar.activation(at[:], ar[:], func=AF.Sigmoid,
                                             bias=ab_s[:, :1])
                        o = fsb.tile([128, DOUT], f32, tag="f2o", name="f2o")
                        nc.scalar.activation(o[:], hh[:], func=AF.Copy,
                                             scale=at[:])
                        nc.sync.dma_start(
                            out_d.ap()[w * 128:(w + 1) * 128, :], o[:])

                    aggregate(t2_full[:], DOUT, flush2a, flush2b)

    nc.compile()
    return nc


# ---------------------------------------------------------------------------
# entry point
# ---------------------------------------------------------------------------


def _make_in_maps(cfg, host, W1, b1, W2, b2, attn_w, attn_b):
    import ml_dtypes
    NC = cfg.NC
    bf16 = ml_dtypes.bfloat16
    ident = np.eye(128, dtype=np.float32)
    giota = np.tile(np.arange(2, 130, dtype=np.float32), (128, 1)).astype(bf16)
    in_maps = []
    for c in range(NC):
        in_maps.append({
            "x_t": host["x_t"][c],
            "w1": np.asarray(W1, np.float32).astype(bf16),
            "w2": np.asarray(W2, np.float32),
            "b1rep": np.tile(np.asarray(b1, np.float32), (128, 1)),
            "b2rep": np.tile(np.asarray(b2, np.float32), (128, 1)),
            "awrep": np.tile(np.asarray(attn_w, np.float32).reshape(1, -1),
                             (128, 1)),
            "abcol": np.full((128, 1),
                             np.asarray(attn_b, np.float32).reshape(-1)[0],
                             np.float32),
            "dinv_slot": host["dinv_slot"][c],
            "ident": ident,
            "giota": giota,
            "idx_all": host["idx_all"][c],
            "dval_all": host["dval_all"][c],
        })
    return in_maps


def run(x, edge_index, W1, b1, W2, b2, attn_w, attn_b, cfg=None,
        backend="hw", trace=False):
    cfg = cfg or FULL
    plan, host = prep(x, edge_index, cfg)
    nc = build(cfg, plan)
    in_maps = _make_in_maps(cfg, host, W1, b1, W2, b2, attn_w, attn_b)

    if backend == "sim":
        from concourse.bass_interp import MultiCoreSim
        sim = MultiCoreSim(nc, num_cores=cfg.NC, trace=False)
        for c, core in enumerate(sim.cores.values()):
            for name, arr in in_maps[c].items():
                core.tensor(name)[:] = arr
        sim.simulate()
        outs = [core.tensor("out_sh").copy() for core in sim.cores.values()]
        exec_ns = None
    else:
        from concourse import bass_utils
        from concourse.bass_interp import get_hw_module
        old = nc.m
        nc.m = get_hw_module(nc.m)
        try:
            res = bass_utils.run_bass_kernel_spmd(
                nc, in_maps, core_ids=list(range(cfg.NC)), trace=trace)
        finally:
            nc.m = old
        outs = [res.results[c]["out_sh"] for c in range(cfg.NC)]
        exec_ns = res.exec_time_ns

    full = np.concatenate(outs, axis=0)  # [TOT, DOUT] in slot order
    out = full[host["pos"]]              # unpermute -> [N, DOUT]
    return np.ascontiguousarray(out), exec_ns


def kernel(x, edge_index, W1, b1, W2, b2, attn_w, attn_b):
    out, _ = run(x, edge_index, W1, b1, W2, b2, attn_w, attn_b,
                 cfg=FULL, backend="hw", trace=False)
    return out
